# revision 4
# baseline (speedup 1.0000x reference)
"""Two-layer GATConv (PyG-style, edge_dim=1, add_self_loops fill='mean') on
8 trn2 NeuronCores.

V2 strategy (aggregate-in-x-space on the PE)
--------------------------------------------
Destinations are partitioned across the 8 cores (degree-sorted, dealt
round-robin).  Each destination-tile of 128 dsts is split into chunks of
D_t consecutive dsts whose slots (in-edges + self-loop, CSR-sorted) fit in
128 partitions.  The host gathers each chunk's *source features* as a
[128 slot, 128 feat] bf16 block (pure data movement), plus a 0/1 mask
locating each slot's (dst, head) column and a per-slot attention score
alpha = a_src[s] + a_dst[d] + w*k_h (linear projections of the inputs).

Device pipeline per layer:
  ACT:  exps = exp(leakyrelu(alpha))              (two big ops, whole core)
  DVE:  psel = mask * exps                        [128 slot, 128*H] per tile
  PE :  agg[f,(j,h)] += xts_c^T @ psel_c          per chunk, disjoint PSUM cols
        zrow[0,(j,h)] = ones^T @ psel             softmax denominators
        zcol[j,h]     = zrow^T (K=1 matmuls)      transpose to dst-major
        out1[j,(h,c)] = (agg_h)^T @ W_h           projection, dst-major out
  DVE:  out = relu(out1 * (1/z) + bias)           fused epilogue
Layer 2 reuses the identical chunk structure with h1 = layer-1 output as
the gathered features (host re-gathers between the two device programs).
"""
import copy
import os

import ml_dtypes
import numpy as np

import concourse.bass as bass
import concourse.mybir as mybir
import concourse.tile as tile
from contextlib import ExitStack
from concourse.bass_utils import run_bass_kernel_spmd

NCORES = 8
P = 128
N = 50000
E = 800000
IN_DIM = 128
NPC = N // NCORES            # 6250 dsts per core
T = (NPC + P - 1) // P       # 49 tiles
NROWS = T * P                # 6272 rows per core (incl pad dsts)
NEG_SLOPE = 0.2

F32 = mybir.dt.float32
BF16 = mybir.dt.bfloat16
NP_BF16 = ml_dtypes.bfloat16

LAST_EXEC_NS = []


# --------------------------------------------------------------------------
# walrus workaround (from baseline): cap sync waits per instruction.
# --------------------------------------------------------------------------
def _split_waits(nc, limit=1):
    sem = nc.alloc_semaphore("wsplit_tmpl_sem")
    tmpl = {}
    for eng_ty, eng in nc.engines.items():
        tmpl[eng_ty] = eng.wait_ge(sem, 0).ins
    tmpl_names = {mi.name for mi in tmpl.values()}
    for f in nc.m.functions:
        for bb in f.blocks:
            insts = [i for i in bb.instructions if i.name not in tmpl_names]
            out = []
            for inst in insts:
                si = inst.sync_info
                waits = list(si.on_wait) if si and si.on_wait else []
                tn = type(inst).__name__
                eff = 0 if (tn == "InstDrain" or "Branch" in tn) else limit
                if len(waits) > eff:
                    head = waits[:-eff] if eff else waits
                    for w in head:
                        c = copy.deepcopy(tmpl[inst.engine])
                        c.name = f"I-wsplit-{nc.next_id()}"
                        c.sync_info = mybir.SyncInfo(on_wait=[w], on_update=[])
                        out.append(c)
                    inst.sync_info = mybir.SyncInfo(
                        on_wait=waits[-eff:] if eff else [],
                        on_update=list(si.on_update) if si.on_update else [],
                    )
                out.append(inst)
            bb.instructions = out


def _ap(root, extra_off, dims):
    return bass.AP(root.tensor, root.offset + extra_off, [list(d) for d in dims])


# --------------------------------------------------------------------------
# device program: one GAT layer
# --------------------------------------------------------------------------
def _build_layer(DT, NCHT, H, C, relu):
    """DT[t]: dsts per chunk; NCHT[t]: chunks per tile. Feature dim = 128."""
    HC = H * C
    PH = P * H                   # psel/mask/agg column count per tile
    NCH = sum(NCHT)
    chb = np.concatenate([[0], np.cumsum(NCHT)])
    odt = BF16 if relu else F32  # L1 output feeds host gather; L2 is final

    nc = bass.Bass()
    xts = nc.dram_tensor("xts", [P, NCH * P], BF16, kind="ExternalInput")
    msk = nc.dram_tensor("msk", [P, T * PH], BF16, kind="ExternalInput")
    asl = nc.dram_tensor("asl", [P, NCH * H], F32, kind="ExternalInput")
    wmat = nc.dram_tensor("wmat", [P, HC], BF16, kind="ExternalInput")
    bvec = nc.dram_tensor("bvec", [P, HC], F32, kind="ExternalInput")
    outp = nc.dram_tensor("out", [NROWS, HC], odt, kind="ExternalOutput")

    with ExitStack() as ctx:
        tc = ctx.enter_context(tile.TileContext(nc))
        pers = ctx.enter_context(tc.tile_pool(name="pers", bufs=1))
        xp = ctx.enter_context(tc.tile_pool(name="xp", bufs=3))
        mp_ = ctx.enter_context(tc.tile_pool(name="mp", bufs=3))
        sb = ctx.enter_context(tc.tile_pool(name="sb", bufs=3))
        ob = ctx.enter_context(tc.tile_pool(name="ob", bufs=3))
        ps = ctx.enter_context(tc.tile_pool(name="ps", bufs=2, space="PSUM"))

        wsb = pers.tile([P, HC], BF16)
        nc.sync.dma_start(out=wsb[:], in_=wmat[:, :])
        bsb = pers.tile([P, HC], F32)
        nc.sync.dma_start(out=bsb[:], in_=bvec[:, :])
        ones1 = pers.tile([P, 1], BF16)
        nc.vector.memset(ones1[:], 1.0)
        one1 = pers.tile([P, 1], BF16)
        nc.vector.memset(one1[:], 1.0)

        # whole-core attention scores -> exp(leakyrelu(.))
        asb = pers.tile([P, NCH * H], F32)
        nc.sync.dma_start(out=asb[:], in_=asl[:, :])
        lrl = pers.tile([P, NCH * H], F32)
        nc.scalar.activation(out=lrl[:], in_=asb[:],
                             func=mybir.ActivationFunctionType.Lrelu,
                             alpha=NEG_SLOPE)
        exps = pers.tile([P, NCH * H], F32)
        nc.scalar.activation(out=exps[:], in_=lrl[:],
                             func=mybir.ActivationFunctionType.Exp)
        e0 = exps[:]
        epitch = e0.ap[0][0]

        for t in range(T):
            D = DT[t]
            nch = NCHT[t]
            cb = int(chb[t])
            xt = xp.tile([P, nch * P], BF16, tag="xt")
            nc.sync.dma_start(out=xt[:], in_=xts[:, cb * P:(cb + nch) * P])
            mt = mp_.tile([P, PH], BF16, tag="mt")
            nc.sync.dma_start(out=mt[:], in_=msk[:, t * PH:(t + 1) * PH])

            # psel = mask * exp(leakyrelu(alpha))  [P, PH]
            psel = sb.tile([P, PH], BF16, tag="psel")
            m0 = mt[:]
            mpitch = m0.ap[0][0]
            p0 = psel[:]
            ppitch = p0.ap[0][0]
            nfull = nch - 1
            Dl = P - D * nfull           # dsts in last chunk
            if nfull > 0:
                o_m = _ap(m0, 0, [(mpitch, P), (H * D, nfull), (0, D), (1, H)])
                o_p = _ap(p0, 0, [(ppitch, P), (H * D, nfull), (0, D), (1, H)])
                i_e = _ap(e0, cb * H, [(epitch, P), (H, nfull), (0, D), (1, H)])
                nc.vector.tensor_tensor(out=o_p, in0=o_m, in1=i_e,
                                        op=mybir.AluOpType.mult)
            off = nfull * D * H
            o_m = _ap(m0, off, [(mpitch, P), (0, Dl), (1, H)])
            o_p = _ap(p0, off, [(ppitch, P), (0, Dl), (1, H)])
            i_e = _ap(e0, (cb + nfull) * H, [(epitch, P), (0, Dl), (1, H)])
            nc.vector.tensor_tensor(out=o_p, in0=o_m, in1=i_e,
                                    op=mybir.AluOpType.mult)

            # chunk matmuls: agg[f, (j,h)] += xts_c^T @ psel_c
            aggps = ps.tile([P, PH], F32, tag="aggps")
            for c in range(nch):
                w0 = c * D * H
                w1 = min((c + 1) * D, P) * H
                nc.tensor.matmul(out=aggps[:, w0:w1],
                                 lhsT=xt[:, c * P:(c + 1) * P],
                                 rhs=psel[:, w0:w1], start=True, stop=True)
            # softmax denominators: zrow[0, (j,h)] = ones^T @ psel
            zps = ps.tile([P, PH], F32, tag="zps")
            nc.tensor.matmul(out=zps[0:1, :], lhsT=ones1[:, 0:1],
                             rhs=psel[:, :], start=True, stop=True)
            zrow = sb.tile([1, PH], BF16, tag="zrow")
            nc.scalar.copy(out=zrow[:], in_=zps[0:1, :])
            # transpose z to dst-major via K=1 matmuls
            zcps = ps.tile([P, H], F32, tag="zcps")
            zr0 = zrow[:]
            for h in range(H):
                nc.tensor.matmul(out=zcps[:, h:h + 1],
                                 lhsT=_ap(zr0, h, [(zr0.ap[0][0], 1), (H, P)]),
                                 rhs=one1[0:1, 0:1], start=True, stop=True)
            zr = sb.tile([P, H], F32, tag="zr")
            nc.vector.reciprocal(out=zr[:], in_=zcps[:, 0:H])

            # copy agg to SBUF (bf16) and project: out1[j, (h,c)]
            aggsb = sb.tile([P, PH], BF16, tag="aggsb")
            nc.scalar.copy(out=aggsb[:], in_=aggps[:])
            o1ps = ps.tile([P, HC], F32, tag="o1ps")
            a0 = aggsb[:]
            apitch = a0.ap[0][0]
            for h in range(H):
                nc.tensor.matmul(out=o1ps[:, h * C:(h + 1) * C],
                                 lhsT=_ap(a0, h, [(apitch, P), (H, P)]),
                                 rhs=wsb[:, h * C:(h + 1) * C],
                                 start=True, stop=True)
            # epilogue: out = [relu](out1 * zr + bias)
            osb = ob.tile([P, HC], odt, tag="osb")
            if relu:
                tmp = sb.tile([P, HC], F32, tag="tmp")
                for h in range(H):
                    nc.vector.scalar_tensor_tensor(
                        out=tmp[:, h * C:(h + 1) * C],
                        in0=o1ps[:, h * C:(h + 1) * C],
                        scalar=zr[:, h:h + 1], in1=bsb[:, h * C:(h + 1) * C],
                        op0=mybir.AluOpType.mult, op1=mybir.AluOpType.add)
                nc.vector.tensor_scalar_max(out=osb[:], in0=tmp[:],
                                            scalar1=0.0)
            else:
                nc.vector.scalar_tensor_tensor(
                    out=osb[:], in0=o1ps[:, :], scalar=zr[:, 0:1],
                    in1=bsb[:, :], op0=mybir.AluOpType.mult,
                    op1=mybir.AluOpType.add)
            nc.sync.dma_start(out=outp[t * P:(t + 1) * P, :], in_=osb[:])

    _split_waits(nc)
    return nc


# --------------------------------------------------------------------------
# host-side planning
# --------------------------------------------------------------------------
def _plan(edge_index):
    src = np.asarray(edge_index[0], dtype=np.int64)
    dst = np.asarray(edge_index[1], dtype=np.int64)
    deg = np.bincount(dst, minlength=N)
    order = np.argsort(-deg, kind="stable")
    rank_of = np.empty(N, np.int64)
    rank_of[order] = np.arange(N)
    core_of = (rank_of % NCORES).astype(np.int64)
    loc_of = (rank_of // NCORES).astype(np.int64)

    # per-tile chunking (same for every core: tiles are degree-homogeneous)
    maxdeg = np.empty(T, np.int64)
    for t in range(T):
        r0 = 1024 * t
        r1 = min(1024 * (t + 1), N)
        maxdeg[t] = deg[order[r0:r1]].max() if r1 > r0 else 0
    DT = np.maximum(P // (maxdeg + 1), 1).astype(np.int64)
    NCHT = ((P + DT - 1) // DT).astype(np.int64)
    chb = np.concatenate([[0], np.cumsum(NCHT)])
    NCH = int(chb[-1])

    # per-dst slot base offsets within its chunk (same structure every core)
    # local dst j in tile t -> chunk c = j // DT[t], base = cumsum of
    # (deg+1) over dsts [c*DT[t], j) of the same core/tile.
    # Edge slot index = base + kpos, self slot = base + deg.
    eorder = np.argsort(dst, kind="stable")
    starts = np.concatenate([[0], np.cumsum(deg)])
    kpos_sorted = np.arange(E) - starts[dst[eorder]]
    kpos = np.empty(E, np.int64)
    kpos[eorder] = kpos_sorted

    # per-core per-local-dst info
    # nodes[c, l] = node at core c local l  (l < NROWS; pads -> -1)
    nodes = -np.ones((NCORES, NROWS), np.int64)
    for c in range(NCORES):
        nn_ = order[c::NCORES]
        nodes[c, :nn_.size] = nn_
    degl = np.where(nodes >= 0, deg[np.clip(nodes, 0, None)], 0)  # [8, NROWS]
    sz = degl + 1                                                  # slot count
    # chunk-local base offset for each local dst
    base = np.zeros((NCORES, NROWS), np.int64)
    jj = np.arange(NROWS)
    tt = jj >> 7
    jrel = jj & 127
    cc = jrel // DT[tt]                       # chunk within tile [NROWS]
    cum = np.cumsum(sz, axis=1)
    prev = np.concatenate([np.zeros((NCORES, 1), np.int64), cum[:, :-1]], 1)
    # subtract cumsum at chunk start
    chunk_id = tt * 1000 + cc                 # unique per (tile, chunk)
    first = np.concatenate([[True], chunk_id[1:] != chunk_id[:-1]])
    fidx = np.maximum.accumulate(np.where(first, jj, 0))
    base = prev - prev[:, fidx]
    # chunk global id per local dst
    cgid = chb[tt] + cc                        # [NROWS]

    return dict(src=src, dst=dst, deg=deg, order=order, core_of=core_of,
                loc_of=loc_of, DT=DT, NCHT=NCHT, chb=chb, NCH=NCH,
                kpos=kpos, nodes=nodes, degl=degl, base=base, cgid=cgid,
                tt=tt, jrel=jrel, cc=cc)


def _build_inputs(plan, feats, alpha_e, alpha_self, W, bias, H, C):
    """Per-core input maps for one layer.

    feats: [N, 128] f32 source features; alpha_e: [E, H] edge scores;
    alpha_self: [N, H] self-loop scores; W: [128, H*C]; bias: [H*C]."""
    DT, NCHT, chb, NCH = plan["DT"], plan["NCHT"], plan["chb"], plan["NCH"]
    src, dst = plan["src"], plan["dst"]
    core_of, loc_of = plan["core_of"], plan["loc_of"]
    kpos = plan["kpos"]
    nodes, degl, base, cgid = (plan["nodes"], plan["degl"], plan["base"],
                               plan["cgid"])
    tt, jrel = plan["tt"], plan["jrel"]
    PH = P * H

    feats_bf = feats.astype(NP_BF16)
    e_core = core_of[dst]
    e_loc = loc_of[dst]                     # local dst index
    e_cg = cgid[e_loc]                      # chunk global id
    e_p = base[e_core, e_loc] + kpos        # slot partition
    e_t = tt[e_loc]
    e_w = (jrel[e_loc] * H)                 # mask column base (j_rel*H)

    maps = []
    for c in range(NCORES):
        m = e_core == c
        # xts [128, NCH*128] bf16 via [NCH, 128, 128] scatter
        x3 = np.zeros((NCH, P, P), NP_BF16)
        x3[e_cg[m], e_p[m]] = feats_bf[src[m]]
        # self slots
        lsel = nodes[c] >= 0
        l_idx = np.nonzero(lsel)[0]
        l_cg = cgid[l_idx]
        l_p = base[c, l_idx] + degl[c, l_idx]
        x3[l_cg, l_p] = feats_bf[nodes[c, l_idx]]
        xts = np.ascontiguousarray(x3.transpose(1, 0, 2).reshape(P, NCH * P))

        # mask [128, T*PH] bf16 via [T, 128, PH]
        m3 = np.zeros((T, P, PH), NP_BF16)
        km = kpos[m]
        for h in range(H):
            m3[e_t[m], e_p[m], e_w[m] + h] = 1.0
        l_t = tt[l_idx]
        l_w = jrel[l_idx] * H
        for h in range(H):
            m3[l_t, l_p, l_w + h] = 1.0
        # pad dsts (nodes == -1): self slot with mask 1, alpha 0, feats 0
        p_idx = np.nonzero(~lsel)[0]
        if p_idx.size:
            p_cg = cgid[p_idx]
            p_p = base[c, p_idx]            # degl = 0 -> self at base
            p_t = tt[p_idx]
            p_w = jrel[p_idx] * H
            for h in range(H):
                m3[p_t, p_p, p_w + h] = 1.0
        mskb = np.ascontiguousarray(m3.transpose(1, 0, 2).reshape(P, T * PH))

        # asl [128, NCH*H] f32
        a3 = np.zeros((NCH, P, H), np.float32)
        a3[e_cg[m], e_p[m]] = alpha_e[m]
        a3[l_cg, l_p] = alpha_self[nodes[c, l_idx]]
        aslb = np.ascontiguousarray(a3.transpose(1, 0, 2).reshape(P, NCH * H))

        maps.append({
            "xts": xts,
            "msk": mskb,
            "asl": aslb,
            "wmat": np.ascontiguousarray(W.astype(NP_BF16)),
            "bvec": np.tile(bias.reshape(1, -1).astype(np.float32), (P, 1)),
        })
    return maps


def _collect(plan, results, key="out"):
    stack = np.stack([np.asarray(r[key], np.float32) for r in results])
    return stack[plan["core_of"], plan["loc_of"], :]


def _alpha(feats, ew_mean, ew, src, dst, W, att_src, att_dst, W_edge,
           att_edge, H, C):
    """Edge and self-loop attention scores (linear projections, host side)."""
    Wa_s = np.stack([W[:, h * C:(h + 1) * C] @ att_src[h] for h in range(H)], 1)
    Wa_d = np.stack([W[:, h * C:(h + 1) * C] @ att_dst[h] for h in range(H)], 1)
    a_src = feats @ Wa_s                     # [N, H]
    a_dst = feats @ Wa_d
    kh = np.array([W_edge[0, h * C:(h + 1) * C] @ att_edge[h]
                   for h in range(H)], np.float32)       # [H]
    alpha_e = a_src[src] + a_dst[dst] + ew[:, None] * kh[None, :]
    alpha_self = a_src + a_dst + ew_mean[:, None] * kh[None, :]
    return alpha_e.astype(np.float32), alpha_self.astype(np.float32)


def kernel(x, edge_index, edge_weight, W1, att_src1, att_dst1, W_edge1,
           att_edge1, b1, W2, att_src2, att_dst2, W_edge2, att_edge2, b2):
    global LAST_EXEC_NS
    LAST_EXEC_NS = []
    trace = os.environ.get("BASSGNN_TRACE", "0") == "1"

    x = np.asarray(x, np.float32)
    ew = np.asarray(edge_weight, np.float32).reshape(-1)
    plan = _plan(np.asarray(edge_index))
    src, dst, deg = plan["src"], plan["dst"], plan["deg"]
    wsum = np.zeros(N, np.float64)
    np.add.at(wsum, dst, ew)
    ew_mean = (wsum / np.maximum(deg, 1)).astype(np.float32)

    core_ids = list(range(NCORES))

    # ---- layer 1 ----
    W1 = np.asarray(W1, np.float32)
    a_e1, a_s1 = _alpha(x, ew_mean, ew, src, dst, W1, np.asarray(att_src1),
                        np.asarray(att_dst1), np.asarray(W_edge1),
                        np.asarray(att_edge1), 2, 64)
    nc1 = _build_layer(plan["DT"], plan["NCHT"], 2, 64, relu=True)
    maps1 = _build_inputs(plan, x, a_e1, a_s1, W1, np.asarray(b1), 2, 64)
    r1 = run_bass_kernel_spmd(nc1, maps1, core_ids, trace=trace)
    if trace:
        LAST_EXEC_NS.append(r1.exec_time_ns)
    h1 = _collect(plan, r1.results)                     # [N, 128] f32

    # ---- layer 2 ----
    W2 = np.asarray(W2, np.float32)
    a_e2, a_s2 = _alpha(h1, ew_mean, ew, src, dst, W2, np.asarray(att_src2),
                        np.asarray(att_dst2), np.asarray(W_edge2),
                        np.asarray(att_edge2), 1, 64)
    nc2 = _build_layer(plan["DT"], plan["NCHT"], 1, 64, relu=False)
    maps2 = _build_inputs(plan, h1, a_e2, a_s2, W2, np.asarray(b2), 1, 64)
    r2 = run_bass_kernel_spmd(nc2, maps2, core_ids, trace=trace)
    if trace:
        LAST_EXEC_NS.append(r2.exec_time_ns)
    return _collect(plan, r2.results).astype(np.float32)


# revision 9
# speedup vs baseline: 2.6859x; 2.6859x over previous
"""Two-layer GATConv (PyG-style, edge_dim=1, add_self_loops fill='mean') on
8 trn2 NeuronCores.

V2 strategy (aggregate-in-x-space on the PE)
--------------------------------------------
Destinations are partitioned across the 8 cores (degree-sorted, dealt
round-robin).  Each destination-tile of 128 dsts is split into chunks of
D_t consecutive dsts whose slots (in-edges + self-loop, CSR-sorted) fit in
128 partitions.  The host gathers each chunk's *source features* as a
[128 slot, 128 feat] bf16 block (pure data movement), plus a 0/1 mask
locating each slot's (dst, head) column and a per-slot attention score
alpha = a_src[s] + a_dst[d] + w*k_h (linear projections of the inputs).

Device pipeline per layer:
  ACT:  exps = exp(leakyrelu(alpha))              (two big ops, whole core)
  DVE:  psel = mask * exps                        [128 slot, 128*H] per tile
  PE :  agg[f,(j,h)] += xts_c^T @ psel_c          per chunk, disjoint PSUM cols
        zrow[0,(j,h)] = ones^T @ psel             softmax denominators
        zcol[j,h]     = zrow^T (K=1 matmuls)      transpose to dst-major
        out1[j,(h,c)] = (agg_h)^T @ W_h           projection, dst-major out
  DVE:  out = relu(out1 * (1/z) + bias)           fused epilogue
Layer 2 reuses the identical chunk structure with h1 = layer-1 output as
the gathered features (host re-gathers between the two device programs).
"""
import copy
import os

import ml_dtypes
import numpy as np

import concourse.bass as bass
import concourse.mybir as mybir
import concourse.tile as tile
from contextlib import ExitStack
from concourse.bass_utils import run_bass_kernel_spmd

NCORES = 8
P = 128
N = 50000
E = 800000
IN_DIM = 128
NPC = N // NCORES            # 6250 dsts per core
T = (NPC + P - 1) // P       # 49 tiles
NROWS = T * P                # 6272 rows per core (incl pad dsts)
NEG_SLOPE = 0.2

F32 = mybir.dt.float32
BF16 = mybir.dt.bfloat16
NP_BF16 = ml_dtypes.bfloat16

LAST_EXEC_NS = []


# --------------------------------------------------------------------------
# walrus workaround (from baseline): cap sync waits per instruction.
# --------------------------------------------------------------------------
def _split_waits(nc, limit=1):
    sem = nc.alloc_semaphore("wsplit_tmpl_sem")
    tmpl = {}
    for eng_ty, eng in nc.engines.items():
        tmpl[eng_ty] = eng.wait_ge(sem, 0).ins
    tmpl_names = {mi.name for mi in tmpl.values()}
    for f in nc.m.functions:
        for bb in f.blocks:
            insts = [i for i in bb.instructions if i.name not in tmpl_names]
            out = []
            for inst in insts:
                si = inst.sync_info
                waits = list(si.on_wait) if si and si.on_wait else []
                tn = type(inst).__name__
                eff = 0 if (tn == "InstDrain" or "Branch" in tn) else limit
                if len(waits) > eff:
                    head = waits[:-eff] if eff else waits
                    for w in head:
                        c = copy.deepcopy(tmpl[inst.engine])
                        c.name = f"I-wsplit-{nc.next_id()}"
                        c.sync_info = mybir.SyncInfo(on_wait=[w], on_update=[])
                        out.append(c)
                    inst.sync_info = mybir.SyncInfo(
                        on_wait=waits[-eff:] if eff else [],
                        on_update=list(si.on_update) if si.on_update else [],
                    )
                out.append(inst)
            bb.instructions = out


def _ap(root, extra_off, dims):
    return bass.AP(root.tensor, root.offset + extra_off, [list(d) for d in dims])


# --------------------------------------------------------------------------
# device program: one GAT layer
# --------------------------------------------------------------------------
def _build_layer(DT, NCHT, H, C, relu, split_waits=True):
    """DT[t]: dsts per chunk; NCHT[t]: chunks per tile. Feature dim = 128."""
    HC = H * C
    PH = P * H                   # psel/mask/agg column count per tile
    NCH = sum(NCHT)
    chb = np.concatenate([[0], np.cumsum(NCHT)])
    odt = BF16 if relu else F32  # L1 output feeds host gather; L2 is final

    nc = bass.Bass()
    xts = nc.dram_tensor("xts", [P, NCH * P], BF16, kind="ExternalInput")
    msk = nc.dram_tensor("msk", [P, T * PH], BF16, kind="ExternalInput")
    asl = nc.dram_tensor("asl", [P, NCH * H], F32, kind="ExternalInput")
    wmat = nc.dram_tensor("wmat", [P, HC], BF16, kind="ExternalInput")
    bvec = nc.dram_tensor("bvec", [P, HC], F32, kind="ExternalInput")
    outp = nc.dram_tensor("out", [NROWS, HC], odt, kind="ExternalOutput")

    with ExitStack() as ctx:
        tc = ctx.enter_context(tile.TileContext(nc))
        pers = ctx.enter_context(tc.tile_pool(name="pers", bufs=1))
        xp = ctx.enter_context(tc.tile_pool(name="xp", bufs=3))
        mp_ = ctx.enter_context(tc.tile_pool(name="mp", bufs=3))
        sb = ctx.enter_context(tc.tile_pool(name="sb", bufs=3))
        ob = ctx.enter_context(tc.tile_pool(name="ob", bufs=3))
        ps = ctx.enter_context(tc.tile_pool(name="ps", bufs=2, space="PSUM"))

        wsb = pers.tile([P, HC], BF16)
        nc.sync.dma_start(out=wsb[:], in_=wmat[:, :])
        bsb = pers.tile([P, HC], F32)
        nc.sync.dma_start(out=bsb[:], in_=bvec[:, :])
        ones1 = pers.tile([P, 1], BF16)
        nc.vector.memset(ones1[:], 1.0)
        one1 = pers.tile([P, 1], BF16)
        nc.vector.memset(one1[:], 1.0)

        # whole-core attention scores -> exp(leakyrelu(.))
        asb = pers.tile([P, NCH * H], F32)
        nc.sync.dma_start(out=asb[:], in_=asl[:, :])
        lrl = pers.tile([P, NCH * H], F32)
        nc.vector.tensor_scalar_mul(out=lrl[:], in0=asb[:], scalar1=NEG_SLOPE)
        nc.vector.tensor_tensor(out=lrl[:], in0=lrl[:], in1=asb[:],
                                op=mybir.AluOpType.max)
        exps = pers.tile([P, NCH * H], F32)
        nc.scalar.activation(out=exps[:], in_=lrl[:],
                             func=mybir.ActivationFunctionType.Exp)
        e0 = exps[:]
        epitch = e0.ap[0][0]

        for t in range(T):
            D = int(DT[t])
            nch = int(NCHT[t])
            cb = int(chb[t])
            xt = xp.tile([P, nch * P], BF16, tag="xt")
            nc.sync.dma_start(out=xt[:], in_=xts[:, cb * P:(cb + nch) * P])
            mt = mp_.tile([P, PH], BF16, tag="mt")
            nc.sync.dma_start(out=mt[:], in_=msk[:, t * PH:(t + 1) * PH])

            # psel = mask * exp(leakyrelu(alpha))  [P, PH]
            psel = sb.tile([P, PH], BF16, tag="psel")
            m0 = mt[:]
            mpitch = m0.ap[0][0]
            p0 = psel[:]
            ppitch = p0.ap[0][0]
            nfull = nch - 1
            Dl = P - D * nfull           # dsts in last chunk
            if nfull > 0:
                o_m = _ap(m0, 0, [(mpitch, P), (H * D, nfull), (H, D), (1, H)])
                o_p = _ap(p0, 0, [(ppitch, P), (H * D, nfull), (H, D), (1, H)])
                i_e = _ap(e0, cb * H, [(epitch, P), (H, nfull), (0, D), (1, H)])
                nc.vector.tensor_tensor(out=o_p, in0=o_m, in1=i_e,
                                        op=mybir.AluOpType.mult)
            off = nfull * D * H
            o_m = _ap(m0, off, [(mpitch, P), (H, Dl), (1, H)])
            o_p = _ap(p0, off, [(ppitch, P), (H, Dl), (1, H)])
            i_e = _ap(e0, (cb + nfull) * H, [(epitch, P), (0, Dl), (1, H)])
            nc.vector.tensor_tensor(out=o_p, in0=o_m, in1=i_e,
                                    op=mybir.AluOpType.mult)

            # chunk matmuls: agg[f, (j,h)] += xts_c^T @ psel_c
            aggps = ps.tile([P, PH], F32, tag="aggps")
            for c in range(nch):
                w0 = c * D * H
                w1 = min((c + 1) * D, P) * H
                nc.tensor.matmul(out=aggps[:, w0:w1],
                                 lhsT=xt[:, c * P:(c + 1) * P],
                                 rhs=psel[:, w0:w1], start=True, stop=True)
            # softmax denominators: zrow[0, (j,h)] = ones^T @ psel
            zps = ps.tile([P, PH], F32, tag="zps")
            nc.tensor.matmul(out=zps[0:1, :], lhsT=ones1[:, 0:1],
                             rhs=psel[:, :], start=True, stop=True)
            zrow = sb.tile([1, PH], BF16, tag="zrow")
            nc.scalar.copy(out=zrow[:], in_=zps[0:1, :])
            # transpose z to dst-major via K=1 matmuls
            zcps = ps.tile([P, H], F32, tag="zcps")
            zr0 = zrow[:]
            for h in range(H):
                nc.tensor.matmul(out=zcps[:, h:h + 1],
                                 lhsT=_ap(zr0, h, [(zr0.ap[0][0], 1), (H, P)]),
                                 rhs=one1[0:1, 0:1], start=True, stop=True)
            zr = sb.tile([P, H], F32, tag="zr")
            nc.vector.reciprocal(out=zr[:], in_=zcps[:, 0:H])

            # copy agg to SBUF (bf16) and project: out1[j, (h,c)]
            aggsb = sb.tile([P, PH], BF16, tag="aggsb")
            nc.scalar.copy(out=aggsb[:], in_=aggps[:])
            o1ps = ps.tile([P, HC], F32, tag="o1ps")
            a0 = aggsb[:]
            apitch = a0.ap[0][0]
            for h in range(H):
                nc.tensor.matmul(out=o1ps[:, h * C:(h + 1) * C],
                                 lhsT=_ap(a0, h, [(apitch, P), (H, P)]),
                                 rhs=wsb[:, h * C:(h + 1) * C],
                                 start=True, stop=True)
            # epilogue: out = [relu](out1 * zr + bias)
            osb = ob.tile([P, HC], odt, tag="osb")
            if relu:
                tmp = sb.tile([P, HC], F32, tag="tmp")
                for h in range(H):
                    nc.vector.scalar_tensor_tensor(
                        out=tmp[:, h * C:(h + 1) * C],
                        in0=o1ps[:, h * C:(h + 1) * C],
                        scalar=zr[:, h:h + 1], in1=bsb[:, h * C:(h + 1) * C],
                        op0=mybir.AluOpType.mult, op1=mybir.AluOpType.add)
                nc.vector.tensor_scalar_max(out=osb[:], in0=tmp[:],
                                            scalar1=0.0)
            else:
                nc.vector.scalar_tensor_tensor(
                    out=osb[:], in0=o1ps[:, :], scalar=zr[:, 0:1],
                    in1=bsb[:, :], op0=mybir.AluOpType.mult,
                    op1=mybir.AluOpType.add)
            nc.sync.dma_start(out=outp[t * P:(t + 1) * P, :], in_=osb[:])

    if split_waits:
        _split_waits(nc)
    return nc


# --------------------------------------------------------------------------
# host-side planning
# --------------------------------------------------------------------------
def _plan(edge_index):
    src = np.asarray(edge_index[0], dtype=np.int64)
    dst = np.asarray(edge_index[1], dtype=np.int64)
    deg = np.bincount(dst, minlength=N)
    order = np.argsort(-deg, kind="stable")
    rank_of = np.empty(N, np.int64)
    rank_of[order] = np.arange(N)
    core_of = (rank_of % NCORES).astype(np.int64)
    loc_of = (rank_of // NCORES).astype(np.int64)

    # per-tile chunking (same for every core: tiles are degree-homogeneous)
    maxdeg = np.empty(T, np.int64)
    for t in range(T):
        r0 = 1024 * t
        r1 = min(1024 * (t + 1), N)
        maxdeg[t] = deg[order[r0:r1]].max() if r1 > r0 else 0
    DT = np.maximum(P // (maxdeg + 1), 1).astype(np.int64)
    NCHT = ((P + DT - 1) // DT).astype(np.int64)
    chb = np.concatenate([[0], np.cumsum(NCHT)])
    NCH = int(chb[-1])

    # per-dst slot base offsets within its chunk (same structure every core)
    # local dst j in tile t -> chunk c = j // DT[t], base = cumsum of
    # (deg+1) over dsts [c*DT[t], j) of the same core/tile.
    # Edge slot index = base + kpos, self slot = base + deg.
    eorder = np.argsort(dst, kind="stable")
    starts = np.concatenate([[0], np.cumsum(deg)])
    kpos_sorted = np.arange(E) - starts[dst[eorder]]
    kpos = np.empty(E, np.int64)
    kpos[eorder] = kpos_sorted

    # per-core per-local-dst info
    # nodes[c, l] = node at core c local l  (l < NROWS; pads -> -1)
    nodes = -np.ones((NCORES, NROWS), np.int64)
    for c in range(NCORES):
        nn_ = order[c::NCORES]
        nodes[c, :nn_.size] = nn_
    degl = np.where(nodes >= 0, deg[np.clip(nodes, 0, None)], 0)  # [8, NROWS]
    sz = degl + 1                                                  # slot count
    # chunk-local base offset for each local dst
    base = np.zeros((NCORES, NROWS), np.int64)
    jj = np.arange(NROWS)
    tt = jj >> 7
    jrel = jj & 127
    cc = jrel // DT[tt]                       # chunk within tile [NROWS]
    cum = np.cumsum(sz, axis=1)
    prev = np.concatenate([np.zeros((NCORES, 1), np.int64), cum[:, :-1]], 1)
    # subtract cumsum at chunk start
    chunk_id = tt * 1000 + cc                 # unique per (tile, chunk)
    first = np.concatenate([[True], chunk_id[1:] != chunk_id[:-1]])
    fidx = np.maximum.accumulate(np.where(first, jj, 0))
    base = prev - prev[:, fidx]
    # chunk global id per local dst
    cgid = chb[tt] + cc                        # [NROWS]

    return dict(src=src, dst=dst, deg=deg, order=order, core_of=core_of,
                loc_of=loc_of, DT=DT, NCHT=NCHT, chb=chb, NCH=NCH,
                kpos=kpos, nodes=nodes, degl=degl, base=base, cgid=cgid,
                tt=tt, jrel=jrel, cc=cc)


def _build_inputs(plan, feats, alpha_e, alpha_self, W, bias, H, C):
    """Per-core input maps for one layer.

    feats: [N, 128] f32 source features; alpha_e: [E, H] edge scores;
    alpha_self: [N, H] self-loop scores; W: [128, H*C]; bias: [H*C]."""
    DT, NCHT, chb, NCH = plan["DT"], plan["NCHT"], plan["chb"], plan["NCH"]
    src, dst = plan["src"], plan["dst"]
    core_of, loc_of = plan["core_of"], plan["loc_of"]
    kpos = plan["kpos"]
    nodes, degl, base, cgid = (plan["nodes"], plan["degl"], plan["base"],
                               plan["cgid"])
    tt, jrel = plan["tt"], plan["jrel"]
    PH = P * H

    feats_bf = feats.astype(NP_BF16)
    e_core = core_of[dst]
    e_loc = loc_of[dst]                     # local dst index
    e_cg = cgid[e_loc]                      # chunk global id
    e_p = base[e_core, e_loc] + kpos        # slot partition
    e_t = tt[e_loc]
    e_w = (jrel[e_loc] * H)                 # mask column base (j_rel*H)

    maps = []
    for c in range(NCORES):
        m = e_core == c
        # xts [128, NCH*128] bf16 via [NCH, 128, 128] scatter
        x3 = np.zeros((NCH, P, P), NP_BF16)
        x3[e_cg[m], e_p[m]] = feats_bf[src[m]]
        # self slots
        lsel = nodes[c] >= 0
        l_idx = np.nonzero(lsel)[0]
        l_cg = cgid[l_idx]
        l_p = base[c, l_idx] + degl[c, l_idx]
        x3[l_cg, l_p] = feats_bf[nodes[c, l_idx]]
        xts = np.ascontiguousarray(x3.transpose(1, 0, 2).reshape(P, NCH * P))

        # mask [128, T*PH] bf16 via [T, 128, PH]
        m3 = np.zeros((T, P, PH), NP_BF16)
        km = kpos[m]
        for h in range(H):
            m3[e_t[m], e_p[m], e_w[m] + h] = 1.0
        l_t = tt[l_idx]
        l_w = jrel[l_idx] * H
        for h in range(H):
            m3[l_t, l_p, l_w + h] = 1.0
        # pad dsts (nodes == -1): self slot with mask 1, alpha 0, feats 0
        p_idx = np.nonzero(~lsel)[0]
        if p_idx.size:
            p_cg = cgid[p_idx]
            p_p = base[c, p_idx]            # degl = 0 -> self at base
            p_t = tt[p_idx]
            p_w = jrel[p_idx] * H
            for h in range(H):
                m3[p_t, p_p, p_w + h] = 1.0
        mskb = np.ascontiguousarray(m3.transpose(1, 0, 2).reshape(P, T * PH))

        # asl [128, NCH*H] f32
        a3 = np.zeros((NCH, P, H), np.float32)
        a3[e_cg[m], e_p[m]] = alpha_e[m]
        a3[l_cg, l_p] = alpha_self[nodes[c, l_idx]]
        aslb = np.ascontiguousarray(a3.transpose(1, 0, 2).reshape(P, NCH * H))

        maps.append({
            "xts": xts,
            "msk": mskb,
            "asl": aslb,
            "wmat": np.ascontiguousarray(W.astype(NP_BF16)),
            "bvec": np.tile(bias.reshape(1, -1).astype(np.float32), (P, 1)),
        })
    return maps


def _collect(plan, results, key="out"):
    stack = np.stack([np.asarray(r[key], np.float32) for r in results])
    return stack[plan["core_of"], plan["loc_of"], :]


def _alpha(feats, ew_mean, ew, src, dst, W, att_src, att_dst, W_edge,
           att_edge, H, C):
    """Edge and self-loop attention scores (linear projections, host side)."""
    Wa_s = np.stack([W[:, h * C:(h + 1) * C] @ att_src[h] for h in range(H)], 1)
    Wa_d = np.stack([W[:, h * C:(h + 1) * C] @ att_dst[h] for h in range(H)], 1)
    a_src = feats @ Wa_s                     # [N, H]
    a_dst = feats @ Wa_d
    kh = np.array([W_edge[0, h * C:(h + 1) * C] @ att_edge[h]
                   for h in range(H)], np.float32)       # [H]
    alpha_e = a_src[src] + a_dst[dst] + ew[:, None] * kh[None, :]
    alpha_self = a_src + a_dst + ew_mean[:, None] * kh[None, :]
    return alpha_e.astype(np.float32), alpha_self.astype(np.float32)


def kernel(x, edge_index, edge_weight, W1, att_src1, att_dst1, W_edge1,
           att_edge1, b1, W2, att_src2, att_dst2, W_edge2, att_edge2, b2):
    global LAST_EXEC_NS
    LAST_EXEC_NS = []
    trace = os.environ.get("BASSGNN_TRACE", "0") == "1"

    x = np.asarray(x, np.float32)
    ew = np.asarray(edge_weight, np.float32).reshape(-1)
    plan = _plan(np.asarray(edge_index))
    src, dst, deg = plan["src"], plan["dst"], plan["deg"]
    wsum = np.zeros(N, np.float64)
    np.add.at(wsum, dst, ew)
    ew_mean = (wsum / np.maximum(deg, 1)).astype(np.float32)

    core_ids = list(range(NCORES))

    # ---- layer 1 ----
    W1 = np.asarray(W1, np.float32)
    a_e1, a_s1 = _alpha(x, ew_mean, ew, src, dst, W1, np.asarray(att_src1),
                        np.asarray(att_dst1), np.asarray(W_edge1),
                        np.asarray(att_edge1), 2, 64)
    nc1 = _build_layer(plan["DT"], plan["NCHT"], 2, 64, relu=True)
    maps1 = _build_inputs(plan, x, a_e1, a_s1, W1, np.asarray(b1), 2, 64)
    r1 = run_bass_kernel_spmd(nc1, maps1, core_ids, trace=trace)
    if trace:
        LAST_EXEC_NS.append(r1.exec_time_ns)
    h1 = _collect(plan, r1.results)                     # [N, 128] f32

    # ---- layer 2 ----
    W2 = np.asarray(W2, np.float32)
    a_e2, a_s2 = _alpha(h1, ew_mean, ew, src, dst, W2, np.asarray(att_src2),
                        np.asarray(att_dst2), np.asarray(W_edge2),
                        np.asarray(att_edge2), 1, 64)
    nc2 = _build_layer(plan["DT"], plan["NCHT"], 1, 64, relu=False)
    maps2 = _build_inputs(plan, h1, a_e2, a_s2, W2, np.asarray(b2), 1, 64)
    r2 = run_bass_kernel_spmd(nc2, maps2, core_ids, trace=trace)
    if trace:
        LAST_EXEC_NS.append(r2.exec_time_ns)
    return _collect(plan, r2.results).astype(np.float32)
